# revision 1
# baseline (speedup 1.0000x reference)
"""Trainium2 Bass kernel for CrossAttention (B=2, N=2048, C=768, H=12).

Sharding: core c -> batch b=c//4, head-group g=c%4 (3 heads each).
Each core computes Q/K/V projections for its heads over the full sequence and
attention; an AllToAll exchanges per-head outputs so each core then computes
the full output projection, residual and LayerNorm for its own 512-row
q-shard.

kernel(**inputs) takes the FULL inputs (setup_inputs() keys) and returns the
full [2, 2048, 768] output.
"""

import sys

for _p in ("/opt/trn_rl_repo",):
    if _p not in sys.path:
        sys.path.insert(0, _p)

import numpy as np

B, N, C = 2, 2048, 768
H = 12
DH = 64
EPS = 1e-5
SCALE = DH ** (-0.5)  # 0.125

NCORES = 8
GROUPS = [[0, 1, 2, 3], [4, 5, 6, 7]]
HPC = 3          # heads per core
CS = HPC * DH    # 192 output-feature slice per core
QS = N // 4      # 512 q rows per core
P = 128

_NC_CACHE = {}

# Wo row permutation: gathered AllToAll order is [per-group heads (3g, 3g+1)]
# then [per-group head 3g+2]; Wo rows must match.
import numpy as _np
WO_PERM = _np.concatenate(
    [_np.arange(192 * g, 192 * g + 128) for g in range(4)]
    + [_np.arange(192 * g + 128, 192 * (g + 1)) for g in range(4)]
)


def _build_nc():
    import concourse.bass as bass
    import concourse.mybir as mybir
    import concourse.tile as tile
    from concourse.tile import add_dep_helper
    from concourse import bacc

    f32 = mybir.dt.float32
    bf16 = mybir.dt.bfloat16
    Alu = mybir.AluOpType
    Act = mybir.ActivationFunctionType

    nc = bacc.Bacc(
        "TRN2",
        target_bir_lowering=False,
        debug=False,
        enable_asserts=True,
        num_devices=NCORES,
    )

    # ---- kernel I/O (per-core shapes; host shards the full problem) ----
    qT = nc.dram_tensor("qT", [C, N], bf16, kind="ExternalInput").ap()
    kT = nc.dram_tensor("kT", [C, N], bf16, kind="ExternalInput").ap()
    vT = nc.dram_tensor("vT", [C, N], bf16, kind="ExternalInput").ap()
    wq = nc.dram_tensor("wq", [C, CS], bf16, kind="ExternalInput").ap()
    wk = nc.dram_tensor("wk", [C, CS], bf16, kind="ExternalInput").ap()
    wv = nc.dram_tensor("wv", [C, CS], bf16, kind="ExternalInput").ap()
    wo = nc.dram_tensor("wo", [C, C], bf16, kind="ExternalInput").ap()
    bq = nc.dram_tensor("bq", [CS], f32, kind="ExternalInput").ap()
    bk = nc.dram_tensor("bk", [CS], f32, kind="ExternalInput").ap()
    bv = nc.dram_tensor("bv", [CS], f32, kind="ExternalInput").ap()
    bo = nc.dram_tensor("bo", [C], f32, kind="ExternalInput").ap()
    gamma = nc.dram_tensor("gamma", [C], f32, kind="ExternalInput").ap()
    beta = nc.dram_tensor("beta", [C], f32, kind="ExternalInput").ap()
    qres = nc.dram_tensor("qres", [QS, C], f32, kind="ExternalInput").ap()
    gsel = nc.dram_tensor("gsel", [2], f32, kind="ExternalInput").ap()
    y = nc.dram_tensor("y", [QS, C], f32, kind="ExternalOutput").ap()

    CI = C // P          # 6 contraction chunks
    NJ = N // 512        # 4 n-chunks of 512
    NM = N // P          # 16 kv-chunks of 128
    VS = DH + 1          # 65: v columns + ones column (denominator row)

    with tile.TileContext(nc) as tc:
        const = tc.alloc_tile_pool(name="const", bufs=1)
        persist = tc.alloc_tile_pool(name="persist", bufs=1)
        rows = tc.alloc_tile_pool(name="rows", bufs=2)
        ppool = tc.alloc_tile_pool(name="ppool", bufs=3)
        small = tc.alloc_tile_pool(name="small", bufs=4)
        dram = tc.alloc_tile_pool(name="dram", bufs=1, space="DRAM")

        # ---- constants ----
        wq_sb = const.tile([P, CI, CS], bf16, name="wq_sb")
        wk_sb = const.tile([P, CI, CS], bf16, name="wk_sb")
        wv_sb = const.tile([P, CI, CS], bf16, name="wv_sb")
        nc.sync.dma_start(wk_sb[:], wk.rearrange("(o p) m -> p o m", p=P))
        nc.sync.dma_start(wq_sb[:], wq.rearrange("(o p) m -> p o m", p=P))
        nc.sync.dma_start(wv_sb[:], wv.rearrange("(o p) m -> p o m", p=P))
        wo_sb = const.tile([P, CI, C], bf16, name="wo_sb")

        bqA = const.tile([P, 1], f32, name="bqA")
        bqB = const.tile([DH, 1], f32, name="bqB")
        bkA = const.tile([P, 1], f32, name="bkA")
        bkB = const.tile([DH, 1], f32, name="bkB")
        nc.sync.dma_start(bkA[:], bk[0:P][:, None])
        nc.sync.dma_start(bkB[:], bk[P:CS][:, None])
        nc.sync.dma_start(bqA[:], bq[0:P][:, None])
        nc.sync.dma_start(bqB[:], bq[P:CS][:, None])
        bv_b = const.tile([P, CS], f32, name="bv_b")
        bo_b = const.tile([P, C], f32, name="bo_b")
        gamma_b = const.tile([P, C], f32, name="gamma_b")
        beta_b = const.tile([P, C], f32, name="beta_b")
        nc.sync.dma_start(bv_b[0:1, :], bv[None, :])
        nc.gpsimd.partition_broadcast(bv_b[:], bv_b[0:1, :])
        qres_sb = const.tile([P, QS // P, C], f32, name="qres_sb")

        # ---- persistent activations ----
        qTa = persist.tile([P, N], bf16, name="qTa")    # heads 0,1
        qTb = persist.tile([DH, N], bf16, name="qTb")   # head 2
        kTa = persist.tile([P, N], bf16, name="kTa")
        kTb = persist.tile([DH, N], bf16, name="kTb")
        vaug = persist.tile([P, NM, HPC * VS], bf16, name="vaug")
        nc.vector.memset(
            vaug.rearrange("p m (h d) -> p m h d", d=VS)[:, :, :, DH : DH + 1], 1.0
        )
        gs = const.tile([1, 2], f32, name="gs")
        s0b = const.tile([P, 1], f32, name="s0b")
        s1b = const.tile([P, 1], f32, name="s1b")
        o_h = [persist.tile([DH, N], f32, name=f"o{h}") for h in range(HPC)]
        o_hb = [persist.tile([DH, N], bf16, name=f"ob{h}") for h in range(HPC)]
        l_sb = persist.tile([1, 1024], f32, name="l_sb")
        r_sb = persist.tile([1, 1024], f32, name="r_sb")
        oG = persist.tile([P, CI, QS], bf16, name="oG")

        a2a_in = dram.tile([2 * NJ, CS, QS], bf16, name="a2a_in")
        a2a_out = dram.tile([2 * NJ, CS, QS], bf16, name="a2a_out")

        # ================= Stage A: projections =================
        with tc.tile_pool(name="ppA", bufs=8, space="PSUM") as ppA:
            # --- K ---
            pk_a = [ppA.tile([P, 512], f32, tag="acc", name=f"pka{j}") for j in range(NJ)]
            pk_b = [ppA.tile([P, 512], f32, tag="acc", name=f"pkb{j}") for j in range(NJ)]
            for i in range(CI):
                k_row = rows.tile([P, N], bf16, tag="row", bufs=6, name="k_row")
                nc.sync.dma_start(k_row[:], kT[P * i : P * (i + 1), :])
                st = dict(start=(i == 0), stop=(i == CI - 1))
                for j in range(NJ):
                    s5 = slice(512 * j, 512 * (j + 1))
                    nc.tensor.matmul(pk_a[j][:], wk_sb[:, i, 0:P], k_row[:, s5], **st)
                    nc.tensor.matmul(pk_b[j][0:DH], wk_sb[:, i, P:CS], k_row[:, s5], **st)
            for j in range(NJ):
                s5 = slice(512 * j, 512 * (j + 1))
                nc.vector.tensor_tensor(
                    kTa[:, s5], pk_a[j][:], bkA.to_broadcast((P, 512)), Alu.add
                )
                nc.vector.tensor_tensor(
                    kTb[:, s5], pk_b[j][0:DH], bkB.to_broadcast((DH, 512)), Alu.add
                )
            # --- Q ---
            pq_a = [ppA.tile([P, 512], f32, tag="acc", name=f"pqa{j}") for j in range(NJ)]
            pq_b = [ppA.tile([P, 512], f32, tag="acc", name=f"pqb{j}") for j in range(NJ)]
            for i in range(CI):
                q_row = rows.tile([P, N], bf16, tag="row", bufs=6, name="q_row")
                nc.sync.dma_start(q_row[:], qT[P * i : P * (i + 1), :])
                st = dict(start=(i == 0), stop=(i == CI - 1))
                for j in range(NJ):
                    s5 = slice(512 * j, 512 * (j + 1))
                    nc.tensor.matmul(pq_a[j][:], wq_sb[:, i, 0:P], q_row[:, s5], **st)
                    nc.tensor.matmul(pq_b[j][0:DH], wq_sb[:, i, P:CS], q_row[:, s5], **st)
            for j in range(NJ):
                s5 = slice(512 * j, 512 * (j + 1))
                nc.vector.tensor_tensor(
                    qTa[:, s5], pq_a[j][:], bqA.to_broadcast((P, 512)), Alu.add
                )
                nc.vector.tensor_tensor(
                    qTb[:, s5], pq_b[j][0:DH], bqB.to_broadcast((DH, 512)), Alu.add
                )
            # --- V (natural layout, accumulated per kv-chunk) ---
            for half in range(2):
                pv = [
                    ppA.tile([P, CS], f32, tag="acc", name=f"pv{half}_{m8}")
                    for m8 in range(8)
                ]
                for i in range(CI):
                    v_half = rows.tile([P, 1024], bf16, tag="vrow", bufs=8, name="v_half")
                    nc.sync.dma_start(
                        v_half[:], vT[P * i : P * (i + 1), 1024 * half : 1024 * (half + 1)]
                    )
                    st = dict(start=(i == 0), stop=(i == CI - 1))
                    for m8 in range(8):
                        nc.tensor.matmul(
                            pv[m8][:, 0:CS],
                            v_half[:, P * m8 : P * (m8 + 1)],
                            wv_sb[:, i, :],
                            **st,
                        )
                for m8 in range(8):
                    m = 8 * half + m8
                    dst = vaug.rearrange("p m (h d) -> p m h d", d=VS)[:, m, :, 0:DH]
                    nc.vector.tensor_tensor(
                        dst,
                        pv[m8][:, 0:CS].rearrange("p (h d) -> p h d", d=DH),
                        bv_b.rearrange("p (h d) -> p h d", d=DH),
                        Alu.add,
                    )

        # ================= Stage B: attention (software-pipelined) ========
        with (
            tc.tile_pool(name="ppS", bufs=2, space="PSUM") as ppS,
            tc.tile_pool(name="ppO", bufs=4, space="PSUM") as ppO,
        ):
            def kq_of(h):
                if h < 2:
                    return kTa[DH * h : DH * (h + 1)], qTa[DH * h : DH * (h + 1)]
                return kTb[0:DH], qTb[0:DH]

            def evict_divide(h, qh, po):
                qbase = 1024 * qh
                for q2 in range(2):
                    s5 = slice(qbase + 512 * q2, qbase + 512 * (q2 + 1))
                    nc.vector.tensor_copy(o_h[h][:, s5], po[q2][0:DH])
                    nc.vector.tensor_copy(
                        l_sb[0:1, 512 * q2 : 512 * (q2 + 1)],
                        po[q2][DH : DH + 1],
                    )
                sq = slice(qbase, qbase + 1024)
                nc.vector.reciprocal_approx_fast(out=r_sb[:], in_=l_sb[:])
                rb = ppool.tile([DH, 1024], f32, tag="rb", bufs=2, name="rb")
                nc.gpsimd.partition_broadcast(rb[:], r_sb[0:1, :])
                nc.vector.tensor_tensor(
                    o_hb[h][:, sq], o_h[h][:, sq], rb[:], Alu.mult
                )

            for qh in range(2):
                qbase = 1024 * qh
                # --- heads 0 & 1 jointly: score MMs row-packed (disjoint
                # PE row groups 0-63 / 64-127 run concurrently) ---
                po2 = {
                    h: [
                        ppO.tile([P, 512], f32, tag="o", name=f"po{qh}_{h}_{q2}")
                        for q2 in range(2)
                    ]
                    for h in (0, 1)
                }
                pts = {0: [None] * NM, 1: [None] * NM}
                for m in range(NM):
                    s_t = {
                        h: ppS.tile([P, 1024], f32, tag="s", name=f"ps{qh}{h}{m}")
                        for h in (0, 1)
                    }
                    for q2 in range(2):
                        for h in (0, 1):
                            k_t, q_t = kq_of(h)
                            nc.tensor.matmul(
                                s_t[h][:, 512 * q2 : 512 * (q2 + 1)],
                                k_t[:, P * m : P * (m + 1)],
                                q_t[:, qbase + 512 * q2 : qbase + 512 * (q2 + 1)],
                                start=True,
                                stop=True,
                            )
                    for h in (0, 1):
                        pt = ppool.tile([P, 1024], bf16, tag="p", bufs=5, name="pt")
                        nc.scalar.activation(pt[:], s_t[h][:], Act.Exp, scale=SCALE)
                        pts[h][m] = pt
                    if m >= 1:
                        for h in (0, 1):
                            for q2 in range(2):
                                nc.tensor.matmul(
                                    po2[h][q2][0:VS],
                                    vaug[:, m - 1, VS * h : VS * (h + 1)],
                                    pts[h][m - 1][:, 512 * q2 : 512 * (q2 + 1)],
                                    start=(m - 1 == 0),
                                    stop=False,
                                )
                            pts[h][m - 1] = None
                for h in (0, 1):
                    for q2 in range(2):
                        nc.tensor.matmul(
                            po2[h][q2][0:VS],
                            vaug[:, NM - 1, VS * h : VS * (h + 1)],
                            pts[h][NM - 1][:, 512 * q2 : 512 * (q2 + 1)],
                            start=False,
                            stop=True,
                        )
                for h in (0, 1):
                    evict_divide(h, qh, po2[h])

            # send heads 0,1 slices early (collective itself runs at the end)
            for r in range(NJ):
                for h in (0, 1):
                    for g2 in range(2):
                        nc.sync.dma_start(
                            a2a_in[NJ * g2 + r, DH * h : DH * (h + 1), :],
                            o_hb[h][:, QS * r : QS * (r + 1)],
                        )

            for qh in range(2):
                qbase = 1024 * qh
                # --- head 2 solo ---
                h = 2
                k_t, q_t = kq_of(h)
                po = [
                    ppO.tile([P, 512], f32, tag="o", name=f"po{qh}_{h}_{q2}")
                    for q2 in range(2)
                ]
                pts2 = [None] * NM
                for m in range(NM):
                    ps = ppS.tile([P, 1024], f32, tag="s", name=f"ps{qh}_{h}_{m}")
                    for q2 in range(2):
                        nc.tensor.matmul(
                            ps[:, 512 * q2 : 512 * (q2 + 1)],
                            k_t[:, P * m : P * (m + 1)],
                            q_t[:, qbase + 512 * q2 : qbase + 512 * (q2 + 1)],
                            start=True,
                            stop=True,
                        )
                    pt = ppool.tile([P, 1024], bf16, tag="p", bufs=5, name="pt")
                    nc.scalar.activation(pt[:], ps[:], Act.Exp, scale=SCALE)
                    pts2[m] = pt
                    if m >= 1:
                        for q2 in range(2):
                            nc.tensor.matmul(
                                po[q2][0:VS],
                                vaug[:, m - 1, VS * h : VS * (h + 1)],
                                pts2[m - 1][:, 512 * q2 : 512 * (q2 + 1)],
                                start=(m - 1 == 0),
                                stop=False,
                            )
                        pts2[m - 1] = None
                for q2 in range(2):
                    nc.tensor.matmul(
                        po[q2][0:VS],
                        vaug[:, NM - 1, VS * h : VS * (h + 1)],
                        pts2[NM - 1][:, 512 * q2 : 512 * (q2 + 1)],
                        start=False,
                        stop=True,
                    )
                evict_divide(h, qh, po)

        # deferred tail-only constant loads (emitted late to keep the
        # startup DMA queue clear for stage A input rows)
        nc.sync.dma_start(wo_sb[:], wo.rearrange("(o p) m -> p o m", p=P))
        nc.sync.dma_start(gs[:], gsel[None, :])
        nc.gpsimd.partition_broadcast(s0b[:], gs[0:1, 0:1])
        nc.gpsimd.partition_broadcast(s1b[:], gs[0:1, 1:2])
        nc.sync.dma_start(bo_b[0:1, :], bo[None, :])
        nc.sync.dma_start(gamma_b[0:1, :], gamma[None, :])
        nc.sync.dma_start(beta_b[0:1, :], beta[None, :])
        nc.gpsimd.partition_broadcast(bo_b[:], bo_b[0:1, :])
        nc.gpsimd.partition_broadcast(gamma_b[:], gamma_b[0:1, :])
        nc.gpsimd.partition_broadcast(beta_b[:], beta_b[0:1, :])
        nc.sync.dma_start(qres_sb[:], qres.rearrange("(t p) c -> p t c", p=P))
        nc.vector.tensor_tensor(
            qres_sb[:],
            qres_sb[:],
            bo_b[:, None, :].to_broadcast((P, QS // P, C)),
            Alu.add,
        )

        # ====== Stage C: AllToAll (all heads) =============================
        for r in range(NJ):
            for g2 in range(2):
                nc.sync.dma_start(
                    a2a_in[NJ * g2 + r, 2 * DH : CS, :],
                    o_hb[2][:, QS * r : QS * (r + 1)],
                )
        nc.gpsimd.collective_compute(
            "AllToAll",
            Alu.bypass,
            replica_groups=[list(range(NCORES))],
            ins=[a2a_in.opt()],
            outs=[a2a_out.opt()],
        )
        nc.sync.dma_start(
            oG[:, 0:4, :],
            a2a_out[0:NJ, 0:P, :].rearrange("r s w -> s r w"),
        )
        oGt1 = rows.tile([P, 4, QS], bf16, tag="row", bufs=6, name="oGt1")
        nc.sync.dma_start(
            oGt1[:],
            a2a_out[NJ : 2 * NJ, 0:P, :].rearrange("r s w -> s r w"),
        )
        nc.vector.tensor_scalar(
            oG[:, 0:4, :], oG[:, 0:4, :], s0b[:], None, Alu.mult
        )
        nc.vector.tensor_scalar(oGt1[:], oGt1[:], s1b[:], None, Alu.mult)
        nc.vector.tensor_tensor(oG[:, 0:4, :], oG[:, 0:4, :], oGt1[:], Alu.add)
        for r2 in range(2):
            nc.sync.dma_start(
                oG[:, 4:6, :].rearrange("(r2 s) o w -> r2 s o w", s=DH)[r2],
                a2a_out[0:NJ, 2 * DH : CS, :].rearrange(
                    "(o r2) s w -> r2 s o w", r2=2
                )[r2],
            )
        oGt2 = rows.tile([P, 2, QS], bf16, tag="vrow", bufs=8, name="oGt2")
        for r2 in range(2):
            nc.sync.dma_start(
                oGt2[:].rearrange("(r2 s) o w -> r2 s o w", s=DH)[r2],
                a2a_out[NJ : 2 * NJ, 2 * DH : CS, :].rearrange(
                    "(o r2) s w -> r2 s o w", r2=2
                )[r2],
            )
        nc.vector.tensor_scalar(
            oG[:, 4:6, :], oG[:, 4:6, :], s0b[:], None, Alu.mult
        )
        nc.vector.tensor_scalar(oGt2[:], oGt2[:], s1b[:], None, Alu.mult)
        nc.vector.tensor_tensor(oG[:, 4:6, :], oG[:, 4:6, :], oGt2[:], Alu.add)

        # ======= Stage D: full Wo (token-major) + residual + LayerNorm ====
        with tc.tile_pool(name="ppD", bufs=2, space="PSUM") as ppD:
            for qt in range(QS // P):
                px = ppD.tile([P, C], f32, tag="d", name=f"px{qt}")
                for ci in range(CI):
                    st = dict(start=(ci == 0), stop=(ci == CI - 1))
                    nc.tensor.matmul(
                        px[:, 0:512],
                        oG[:, ci, P * qt : P * (qt + 1)],
                        wo_sb[:, ci, 0:512],
                        **st,
                    )
                    nc.tensor.matmul(
                        px[:, 512:C],
                        oG[:, ci, P * qt : P * (qt + 1)],
                        wo_sb[:, ci, 512:C],
                        **st,
                    )
                x1 = ppool.tile([P, C], f32, tag="x1", bufs=2, name="x1")
                nc.vector.tensor_tensor(x1[:], px[:], qres_sb[:, qt], Alu.add)
                mu = small.tile([P, 1], f32, tag="st", name="mu")
                sq = ppool.tile([P, C], f32, tag="sq", bufs=2, name="sq")
                sqs = small.tile([P, 1], f32, tag="st", name="sqs")
                var = small.tile([P, 1], f32, tag="st", name="var")
                rinv = small.tile([P, 1], f32, tag="st", name="rinv")
                rstd = small.tile([P, 1], f32, tag="st", name="rstd")
                nb = small.tile([P, 1], f32, tag="st", name="nb")
                nc.vector.reduce_sum(mu[:], x1[:], axis=mybir.AxisListType.X)
                nc.vector.tensor_scalar_mul(mu[:], mu[:], 1.0 / C)
                nc.scalar.activation(sq[:], x1[:], Act.Square, accum_out=sqs[:])
                nc.vector.tensor_scalar_mul(sqs[:], sqs[:], 1.0 / C)
                nc.vector.tensor_tensor(var[:], mu[:], mu[:], Alu.mult)
                nc.vector.tensor_tensor(var[:], sqs[:], var[:], Alu.subtract)
                nc.vector.tensor_scalar_add(var[:], var[:], EPS)
                nc.vector.reciprocal(rinv[:], var[:])
                nc.scalar.activation(rstd[:], rinv[:], Act.Sqrt)
                nc.vector.tensor_tensor(nb[:], mu[:], rstd[:], Alu.mult)
                nc.vector.tensor_scalar_mul(nb[:], nb[:], -1.0)
                nc.vector.tensor_scalar(
                    x1[:], x1[:], rstd[:], nb[:], Alu.mult, Alu.add
                )
                nc.vector.tensor_tensor(x1[:], x1[:], gamma_b[:], Alu.mult)
                nc.vector.tensor_tensor(x1[:], x1[:], beta_b[:], Alu.add)
                nc.sync.dma_start(
                    y.rearrange("(t p) c -> p t c", p=P)[:, qt], x1[:]
                )

        for pool in (dram, small, ppool, rows, persist, const):
            pool.release()

    nc.compile()
    return nc


def get_nc():
    if "nc" not in _NC_CACHE:
        _NC_CACHE["nc"] = _build_nc()
    return _NC_CACHE["nc"]


def make_in_maps(inputs):
    import ml_dtypes

    b16 = ml_dtypes.bfloat16
    q = np.asarray(inputs["query"], np.float32)
    k = np.asarray(inputs["key_in"], np.float32)
    v = np.asarray(inputs["value"], np.float32)
    Wq = np.asarray(inputs["Wq"], np.float32)
    Wk = np.asarray(inputs["Wk"], np.float32)
    Wv = np.asarray(inputs["Wv"], np.float32)
    Wo = np.asarray(inputs["Wo"], np.float32)
    bq = np.asarray(inputs["bq"], np.float32)
    bk = np.asarray(inputs["bk"], np.float32)
    bv = np.asarray(inputs["bv"], np.float32)
    bo = np.asarray(inputs["bo"], np.float32)
    gamma = np.asarray(inputs["gamma"], np.float32)
    beta = np.asarray(inputs["beta"], np.float32)

    in_maps = []
    for c in range(NCORES):
        b, g = c // 4, c % 4
        cs = slice(CS * g, CS * (g + 1))
        in_maps.append(
            {
                "qT": np.ascontiguousarray(q[b].T).astype(b16),
                "kT": np.ascontiguousarray(k[b].T).astype(b16),
                "vT": np.ascontiguousarray(v[b].T).astype(b16),
                "wq": np.ascontiguousarray(Wq[:, cs]).astype(b16),
                "wk": np.ascontiguousarray(Wk[:, cs]).astype(b16),
                "wv": np.ascontiguousarray(Wv[:, cs]).astype(b16),
                "wo": Wo[WO_PERM, :].astype(b16),
                "bq": np.ascontiguousarray(bq[cs]),
                "bk": np.ascontiguousarray(bk[cs]),
                "bv": np.ascontiguousarray(bv[cs]),
                "bo": bo.copy(),
                "gamma": gamma.copy(),
                "beta": beta.copy(),
                "qres": np.ascontiguousarray(q[b, QS * g : QS * (g + 1)]),
                "gsel": np.array([1.0 - b, float(b)], np.float32),
            }
        )
    return in_maps


def _install_ntff_shim():
    """Provide antenv.axon_hooks if the image lacks it (needed for trace=True)."""
    try:
        import antenv.axon_hooks  # noqa: F401

        return
    except ImportError:
        pass
    import contextlib
    import ctypes
    import types

    so_path = "/opt/axon/libaxon_pjrt.so"
    state = {"hook": None}

    def set_axon_ntff_profile_hook(h):
        state["hook"] = h

    def get_axon_ntff_profile_hook():
        if state["hook"] is None:
            try:
                lib = ctypes.CDLL(so_path)
            except OSError:
                return None
            if not hasattr(lib, "axon_start_nrt_profile"):
                return None
            lib.axon_start_nrt_profile.argtypes = [
                ctypes.POINTER(ctypes.c_int64),
                ctypes.c_size_t,
            ]
            lib.axon_start_nrt_profile.restype = ctypes.c_int64
            lib.axon_stop_nrt_profile.argtypes = [ctypes.c_char_p]
            lib.axon_stop_nrt_profile.restype = ctypes.c_int64

            @contextlib.contextmanager
            def _hook(output_dir, device_ids):
                import jax

                jax.devices()
                if device_ids:
                    ids = (ctypes.c_int64 * len(device_ids))(*device_ids)
                    rc = lib.axon_start_nrt_profile(ids, len(device_ids))
                else:
                    rc = lib.axon_start_nrt_profile(None, 0)
                if rc != 0:
                    raise RuntimeError(f"axon_start_nrt_profile rc={rc}")
                try:
                    yield
                finally:
                    n = lib.axon_stop_nrt_profile(str(output_dir).encode())
                    print(f"profile: {n} file(s) written to {output_dir}")

            state["hook"] = _hook
        return state["hook"]

    mod = types.ModuleType("antenv.axon_hooks")
    mod.set_axon_ntff_profile_hook = set_axon_ntff_profile_hook
    mod.get_axon_ntff_profile_hook = get_axon_ntff_profile_hook
    import antenv

    antenv.axon_hooks = mod
    sys.modules["antenv.axon_hooks"] = mod


def run(inputs, trace=False, trace_cores=None):
    if trace:
        _install_ntff_shim()
    from concourse.bass_utils import run_bass_kernel_spmd

    nc = get_nc()
    in_maps = make_in_maps(inputs)
    res = run_bass_kernel_spmd(
        nc,
        in_maps,
        list(range(NCORES)),
        trace=trace,
        **({"trace_cores": trace_cores} if trace_cores is not None else {}),
    )
    out = np.empty((B, N, C), np.float32)
    for c in range(NCORES):
        b, g = c // 4, c % 4
        out[b, QS * g : QS * (g + 1)] = res.results[c]["y"]
    return out, res


def kernel(**inputs):
    out, _ = run(inputs, trace=False)
    return out



# revision 6
# speedup vs baseline: 1.0405x; 1.0405x over previous
"""Trainium2 Bass kernel for CrossAttention (B=2, N=2048, C=768, H=12).

Sharding: core c -> batch b=c//4, head-group g=c%4 (3 heads each).
Each core computes Q/K/V projections for its heads over the full sequence and
attention; an AllToAll exchanges per-head outputs so each core then computes
the full output projection, residual and LayerNorm for its own 512-row
q-shard.

v2 schedule: single fused region.  K-proj warms the PE, Q-proj is emitted
just-in-time per 512-column q-chunk and V-proj just-in-time per kv-block so
projection matmuls fill the PE bubbles of the scalar(exp)-bound attention
loop (keeps the HAM clock-gate at 8/8).  The AllToAll is split in two:
heads {0,1} fire after their attention finishes and transfer under head-2's
attention; only the small head-2 AllToAll plus the tail of the output
projection is exposed.

kernel(**inputs) takes the FULL inputs (setup_inputs() keys) and returns the
full [2, 2048, 768] output.
"""

import sys

for _p in ("/opt/trn_rl_repo",):
    if _p not in sys.path:
        sys.path.insert(0, _p)

import numpy as np

B, N, C = 2, 2048, 768
H = 12
DH = 64
EPS = 1e-5
SCALE = DH ** (-0.5)  # 0.125

NCORES = 8
HPC = 3          # heads per core
CS = HPC * DH    # 192 output-feature slice per core
QS = N // 4      # 512 q rows per core
P = 128

_NC_CACHE = {}

# Wo row permutation: gathered order is [per-group heads (3g, 3g+1)] then
# [per-group head 3g+2]; Wo rows must match.
import numpy as _np
WO_PERM = _np.concatenate(
    [_np.arange(192 * g, 192 * g + 128) for g in range(4)]
    + [_np.arange(192 * g + 128, 192 * (g + 1)) for g in range(4)]
)


def _build_nc():
    import concourse.bass as bass
    import concourse.mybir as mybir
    import concourse.tile as tile
    from concourse import bacc

    f32 = mybir.dt.float32
    bf16 = mybir.dt.bfloat16
    Alu = mybir.AluOpType
    Act = mybir.ActivationFunctionType

    nc = bacc.Bacc(
        "TRN2",
        target_bir_lowering=False,
        debug=False,
        enable_asserts=True,
        num_devices=NCORES,
    )

    # ---- kernel I/O (per-core shapes; host shards the full problem) ----
    qT = nc.dram_tensor("qT", [C, N], bf16, kind="ExternalInput").ap()
    kT = nc.dram_tensor("kT", [C, N], bf16, kind="ExternalInput").ap()
    vT = nc.dram_tensor("vT", [C, N], bf16, kind="ExternalInput").ap()
    wq = nc.dram_tensor("wq", [C, CS], bf16, kind="ExternalInput").ap()
    wk = nc.dram_tensor("wk", [C, CS], bf16, kind="ExternalInput").ap()
    wv = nc.dram_tensor("wv", [C, CS], bf16, kind="ExternalInput").ap()
    wo = nc.dram_tensor("wo", [C, C], bf16, kind="ExternalInput").ap()
    bq = nc.dram_tensor("bq", [CS], f32, kind="ExternalInput").ap()
    bk = nc.dram_tensor("bk", [CS], f32, kind="ExternalInput").ap()
    bv = nc.dram_tensor("bv", [CS], f32, kind="ExternalInput").ap()
    bo = nc.dram_tensor("bo", [C], f32, kind="ExternalInput").ap()
    gamma = nc.dram_tensor("gamma", [C], f32, kind="ExternalInput").ap()
    beta = nc.dram_tensor("beta", [C], f32, kind="ExternalInput").ap()
    qres = nc.dram_tensor("qres", [QS, C], f32, kind="ExternalInput").ap()
    gsel = nc.dram_tensor("gsel", [2], f32, kind="ExternalInput").ap()
    y = nc.dram_tensor("y", [QS, C], f32, kind="ExternalOutput").ap()

    CI = C // P          # 6 contraction chunks
    NJ = N // 512        # 4 q-chunks of 512
    NM = N // P          # 16 kv-chunks of 128
    VS = DH + 1          # 65: v columns + ones column (denominator row)
    QT = QS // P         # 4 output row-blocks of 128

    with tile.TileContext(nc) as tc:
        const = tc.alloc_tile_pool(name="const", bufs=1)
        persist = tc.alloc_tile_pool(name="persist", bufs=1)
        rows = tc.alloc_tile_pool(name="rows", bufs=2)
        ppool = tc.alloc_tile_pool(name="ppool", bufs=3)
        small = tc.alloc_tile_pool(name="small", bufs=4)
        dram = tc.alloc_tile_pool(name="dram", bufs=1, space="DRAM")

        # ---------- constants & inputs, DMA emission order = priority ----
        wk_sb = const.tile([P, CI, CS], bf16, name="wk_sb")
        nc.sync.dma_start(wk_sb[:], wk.rearrange("(o p) m -> p o m", p=P))
        bkA = const.tile([P, 1], f32, name="bkA")
        bkB = const.tile([DH, 1], f32, name="bkB")
        nc.sync.dma_start(bkA[:], bk[0:P][:, None])
        nc.sync.dma_start(bkB[:], bk[P:CS][:, None])

        # kT rows (the critical startup path for K-proj)
        k_rows = []
        for i in range(CI):
            kr = rows.tile([P, N], bf16, tag="krow", bufs=6, name=f"k_row{i}")
            nc.sync.dma_start(kr[:], kT[P * i : P * (i + 1), :])
            k_rows.append(kr)

        wq_sb = const.tile([P, CI, CS], bf16, name="wq_sb")
        nc.sync.dma_start(wq_sb[:], wq.rearrange("(o p) m -> p o m", p=P))
        bqA = const.tile([P, 1], f32, name="bqA")
        bqB = const.tile([DH, 1], f32, name="bqB")
        nc.sync.dma_start(bqA[:], bq[0:P][:, None])
        nc.sync.dma_start(bqB[:], bq[P:CS][:, None])

        # qT rows: first 512 columns early (Q-proj r=0), rest after
        q_rows = []
        for i in range(CI):
            qr = persist.tile([P, N], bf16, name=f"q_row{i}")
            nc.sync.dma_start(qr[:, 0:512], qT[P * i : P * (i + 1), 0:512])
            q_rows.append(qr)

        wv_sb = const.tile([P, CI, CS], bf16, name="wv_sb")
        nc.sync.dma_start(wv_sb[:], wv.rearrange("(o p) m -> p o m", p=P))
        bv_b = const.tile([P, CS], f32, name="bv_b")
        nc.sync.dma_start(bv_b[0:1, :], bv[None, :])
        nc.gpsimd.partition_broadcast(bv_b[:], bv_b[0:1, :])

        # vT rows (V-proj JIT slices these from SBUF, no per-block DMA)
        v_rows = []
        for i in range(CI):
            vr = rows.tile([P, N], bf16, tag="vrow", bufs=6, name=f"v_row{i}")
            nc.sync.dma_start(vr[:], vT[P * i : P * (i + 1), :])
            v_rows.append(vr)

        for i in range(CI):
            nc.sync.dma_start(
                q_rows[i][:, 512:N], qT[P * i : P * (i + 1), 512:N]
            )

        # ---------- persistent activations ----------
        qTa = persist.tile([P, N], bf16, name="qTa")    # heads 0,1
        qTb = persist.tile([DH, N], bf16, name="qTb")   # head 2
        kTa = persist.tile([P, N], bf16, name="kTa")
        kTb = persist.tile([DH, N], bf16, name="kTb")
        vaug = persist.tile([P, NM, HPC * VS], bf16, name="vaug")
        nc.vector.memset(
            vaug.rearrange("p m (h d) -> p m h d", d=VS)[:, :, :, DH : DH + 1], 1.0
        )
        o_hb = [persist.tile([DH, N], bf16, name=f"ob{h}") for h in range(HPC)]
        oG = persist.tile([P, CI, QS], bf16, name="oG")

        wo_sb = const.tile([P, CI, C], bf16, name="wo_sb")
        gs = const.tile([1, 2], f32, name="gs")
        s0b = const.tile([P, 1], f32, name="s0b")
        s1b = const.tile([P, 1], f32, name="s1b")
        bo_b = const.tile([P, C], f32, name="bo_b")
        gamma_b = const.tile([P, C], f32, name="gamma_b")
        beta_b = const.tile([P, C], f32, name="beta_b")
        qres_sb = const.tile([P, QT, C], f32, name="qres_sb")

        a2a1_in = dram.tile([2 * NJ, P, QS], bf16, name="a2a1_in")
        a2a1_out = dram.tile([2 * NJ, P, QS], bf16, name="a2a1_out")
        a2a2_in = dram.tile([2 * NJ, DH, QS], bf16, name="a2a2_in")
        a2a2_out = dram.tile([2 * NJ, DH, QS], bf16, name="a2a2_out")

        # ================= K projection (warms the PE) =================
        with tc.tile_pool(name="ppK", bufs=8, space="PSUM") as ppK:
            pk_a = [ppK.tile([P, 512], f32, tag="acc", name=f"pka{j}") for j in range(NJ)]
            pk_b = [ppK.tile([P, 512], f32, tag="acc", name=f"pkb{j}") for j in range(NJ)]
            for i in range(CI):
                st = dict(start=(i == 0), stop=(i == CI - 1))
                for j in range(NJ):
                    s5 = slice(512 * j, 512 * (j + 1))
                    nc.tensor.matmul(pk_a[j][:], wk_sb[:, i, 0:P], k_rows[i][:, s5], **st)
                    nc.tensor.matmul(pk_b[j][0:DH], wk_sb[:, i, P:CS], k_rows[i][:, s5], **st)
            for j in range(NJ):
                s5 = slice(512 * j, 512 * (j + 1))
                nc.vector.tensor_tensor(
                    kTa[:, s5], pk_a[j][:], bkA.to_broadcast((P, 512)), Alu.add
                )
                nc.vector.tensor_tensor(
                    kTb[:, s5], pk_b[j][0:DH], bkB.to_broadcast((DH, 512)), Alu.add
                )

        # ============== fused Q/V-proj + attention region ==============
        def q_proj(r):
            """Project q columns [512r, 512r+512) into qTa/qTb."""
            s5 = slice(512 * r, 512 * (r + 1))
            pq_a = ppF.tile([P, 512], f32, tag="fill", name=f"pqa{r}")
            for i in range(CI):
                nc.tensor.matmul(
                    pq_a[:], wq_sb[:, i, 0:P], q_rows[i][:, s5],
                    start=(i == 0), stop=(i == CI - 1),
                )
            nc.vector.tensor_tensor(
                qTa[:, s5], pq_a[:], bqA.to_broadcast((P, 512)), Alu.add
            )
            pq_b = ppF.tile([P, 512], f32, tag="fill", name=f"pqb{r}")
            for i in range(CI):
                nc.tensor.matmul(
                    pq_b[0:DH], wq_sb[:, i, P:CS], q_rows[i][:, s5],
                    start=(i == 0), stop=(i == CI - 1),
                )
            nc.vector.tensor_tensor(
                qTb[:, s5], pq_b[0:DH], bqB.to_broadcast((DH, 512)), Alu.add
            )

        def v_proj(m):
            """Project kv-block m into vaug[:, m, :]."""
            pv = ppF.tile([P, 512], f32, tag="fill", name=f"pv{m}")
            for i in range(CI):
                nc.tensor.matmul(
                    pv[:, 0:CS], v_rows[i][:, P * m : P * (m + 1)], wv_sb[:, i, :],
                    start=(i == 0), stop=(i == CI - 1),
                )
            dst = vaug.rearrange("p m (h d) -> p m h d", d=VS)[:, m, :, 0:DH]
            nc.vector.tensor_tensor(
                dst,
                pv[:, 0:CS].rearrange("p (h d) -> p h d", d=DH),
                bv_b.rearrange("p (h d) -> p h d", d=DH),
                Alu.add,
            )

        def evict_head(h, r, po):
            """po [65, 512] -> o_hb[h][:, 512r:512r+512] divided by denom."""
            s5 = slice(512 * r, 512 * (r + 1))
            l_t = small.tile([1, 512], f32, tag="lt", name=f"l{h}{r}")
            nc.vector.tensor_copy(l_t[:], po[DH : DH + 1, :])
            r_t = small.tile([1, 512], f32, tag="lt", name=f"r{h}{r}")
            nc.vector.reciprocal_approx_fast(out=r_t[:], in_=l_t[:])
            rb = ppool.tile([DH, 512], f32, tag="rb", bufs=2, name=f"rb{h}{r}")
            nc.gpsimd.partition_broadcast(rb[:], r_t[:])
            nc.vector.tensor_tensor(o_hb[h][:, s5], po[0:DH, :], rb[:], Alu.mult)

        with tc.tile_pool(name="ppO", bufs=1, space="PSUM") as ppO:
            po_h = {
                0: ppO.tile([P, 512], f32, tag="po0", name="po0"),
                1: ppO.tile([P, 512], f32, tag="po1", name="po1"),
            }
            # ---------------- heads 0,1 (row-packed scores) ----------------
            with (
                tc.tile_pool(name="ppS", bufs=2, space="PSUM") as ppS,
                tc.tile_pool(name="ppF", bufs=2, space="PSUM") as ppF,
            ):
                q_proj(0)
                v_proj(0)
                v_proj(1)
                for r in range(NJ):
                    sq = slice(512 * r, 512 * (r + 1))
                    pts = [None] * NM
                    for m in range(NM):
                        sm = slice(P * m, P * (m + 1))
                        s_t = ppS.tile([P, 1024], f32, tag="s", name=f"s{r}_{m}")
                        nc.tensor.matmul(
                            s_t[:, 0:512], kTa[0:DH, sm], qTa[0:DH, sq],
                            start=True, stop=True,
                        )
                        nc.tensor.matmul(
                            s_t[:, 512:1024], kTa[DH:P, sm], qTa[DH:P, sq],
                            start=True, stop=True,
                        )
                        pt = ppool.tile([P, 1024], bf16, tag="p", bufs=4, name="pt")
                        nc.scalar.activation(pt[:], s_t[:], Act.Exp, scale=SCALE)
                        pts[m] = pt
                        # JIT fillers for the PE while exp runs
                        if r == 0 and m + 2 < NM:
                            v_proj(m + 2)
                        if m >= 1:
                            for h in (0, 1):
                                nc.tensor.matmul(
                                    po_h[h][0:VS],
                                    vaug[:, m - 1, VS * h : VS * (h + 1)],
                                    pts[m - 1][:, 512 * h : 512 * (h + 1)],
                                    start=(m - 1 == 0), stop=False,
                                )
                            pts[m - 1] = None
                    for h in (0, 1):
                        nc.tensor.matmul(
                            po_h[h][0:VS],
                            vaug[:, NM - 1, VS * h : VS * (h + 1)],
                            pts[NM - 1][:, 512 * h : 512 * (h + 1)],
                            start=False, stop=True,
                        )
                    for h in (0, 1):
                        evict_head(h, r, po_h[h])
                    for h in (0, 1):
                        for g2 in range(2):
                            nc.sync.dma_start(
                                a2a1_in[NJ * g2 + r, DH * h : DH * (h + 1), :],
                                o_hb[h][:, sq],
                            )
                    if r + 1 < NJ:
                        q_proj(r + 1)

            nc.gpsimd.collective_compute(
                "AllToAll",
                Alu.bypass,
                replica_groups=[list(range(NCORES))],
                ins=[a2a1_in.opt()],
                outs=[a2a1_out.opt()],
            )

            # constants needed by the tail (emitted here to keep the startup
            # DMA queue clear)
            nc.sync.dma_start(wo_sb[:], wo.rearrange("(o p) m -> p o m", p=P))
            nc.sync.dma_start(gs[:], gsel[None, :])
            nc.gpsimd.partition_broadcast(s0b[:], gs[0:1, 0:1])
            nc.gpsimd.partition_broadcast(s1b[:], gs[0:1, 1:2])
            nc.sync.dma_start(bo_b[0:1, :], bo[None, :])
            nc.sync.dma_start(gamma_b[0:1, :], gamma[None, :])
            nc.sync.dma_start(beta_b[0:1, :], beta[None, :])
            nc.gpsimd.partition_broadcast(bo_b[:], bo_b[0:1, :])
            nc.gpsimd.partition_broadcast(gamma_b[:], gamma_b[0:1, :])
            nc.gpsimd.partition_broadcast(beta_b[:], beta_b[0:1, :])
            nc.sync.dma_start(qres_sb[:], qres.rearrange("(t p) c -> p t c", p=P))
            nc.vector.tensor_tensor(
                qres_sb[:],
                qres_sb[:],
                bo_b[:, None, :].to_broadcast((P, QT, C)),
                Alu.add,
            )

            # ---------------- head 2 + early out-proj ----------------
            with (
                tc.tile_pool(name="ppS2", bufs=1, space="PSUM") as ppS2,
                tc.tile_pool(name="ppD", bufs=2, space="PSUM") as ppD,
            ):
                h = 2
                po2 = ppO.tile([P, 512], f32, tag="po0", name="po2")
                px = {}

                def d_partial(qt, ci_list, start_first):
                    if qt not in px:
                        px[qt] = ppD.tile([P, C], f32, tag="px", name=f"px{qt}")
                    for idx, ci in enumerate(ci_list):
                        st = dict(
                            start=(start_first and idx == 0),
                            stop=(ci == CI - 1),
                        )
                        nc.tensor.matmul(
                            px[qt][:, 0:512],
                            oG[:, ci, P * qt : P * (qt + 1)],
                            wo_sb[:, ci, 0:512],
                            **st,
                        )
                        nc.tensor.matmul(
                            px[qt][:, 512:C],
                            oG[:, ci, P * qt : P * (qt + 1)],
                            wo_sb[:, ci, 512:C],
                            **st,
                        )

                for r in range(NJ):
                    sq = slice(512 * r, 512 * (r + 1))
                    if r == 3:
                        # a2a1 finished long ago: assemble oG ci 0..3
                        oGt1 = rows.tile([P, NJ, QS], bf16, tag="krow", bufs=6, name="oGt1")
                        nc.sync.dma_start(
                            oG[:, 0:NJ, :],
                            a2a1_out[0:NJ, :, :].rearrange("r s w -> s r w"),
                        )
                        nc.sync.dma_start(
                            oGt1[:],
                            a2a1_out[NJ : 2 * NJ, :, :].rearrange("r s w -> s r w"),
                        )
                        nc.vector.tensor_scalar(
                            oG[:, 0:NJ, :], oG[:, 0:NJ, :], s0b[:], None, Alu.mult
                        )
                        nc.vector.tensor_scalar(oGt1[:], oGt1[:], s1b[:], None, Alu.mult)
                        nc.vector.tensor_tensor(
                            oG[:, 0:NJ, :], oG[:, 0:NJ, :], oGt1[:], Alu.add
                        )
                    pt2s = [None, None]
                    for mp in range(NM // 2):
                        s_t = ppS2.tile([P, 1024], f32, tag="s2", name=f"t{r}_{mp}")
                        for q2 in range(2):
                            m = 2 * mp + q2
                            nc.tensor.matmul(
                                s_t[:, 512 * q2 : 512 * (q2 + 1)],
                                kTb[0:DH, P * m : P * (m + 1)],
                                qTb[0:DH, sq],
                                start=True, stop=True,
                            )
                        pt2 = ppool.tile([P, 1024], bf16, tag="p", bufs=4, name="pt2")
                        nc.scalar.activation(pt2[:], s_t[:], Act.Exp, scale=SCALE)
                        pt2s[mp % 2] = pt2
                        if mp >= 1:
                            prev = pt2s[(mp - 1) % 2]
                            for q2 in range(2):
                                m = 2 * (mp - 1) + q2
                                nc.tensor.matmul(
                                    po2[0:VS],
                                    vaug[:, m, VS * h : VS * (h + 1)],
                                    prev[:, 512 * q2 : 512 * (q2 + 1)],
                                    start=(m == 0), stop=False,
                                )
                        # early out-proj filler once oG ci0..3 is ready
                        if r == 3 and mp >= 2:
                            if mp < 6:
                                d_partial(0, [mp - 2], mp == 2)
                            else:
                                d_partial(1, [mp - 6], mp == 6)
                    prev = pt2s[(NM // 2 - 1) % 2]
                    for q2 in range(2):
                        m = NM - 2 + q2
                        nc.tensor.matmul(
                            po2[0:VS],
                            vaug[:, m, VS * h : VS * (h + 1)],
                            prev[:, 512 * q2 : 512 * (q2 + 1)],
                            start=False, stop=(m == NM - 1),
                        )
                    evict_head(2, r, po2)
                    for g2 in range(2):
                        nc.sync.dma_start(
                            a2a2_in[NJ * g2 + r, :, :], o_hb[2][:, sq]
                        )

                nc.gpsimd.collective_compute(
                    "AllToAll",
                    Alu.bypass,
                    replica_groups=[list(range(NCORES))],
                    ins=[a2a2_in.opt()],
                    outs=[a2a2_out.opt()],
                )
                d_partial(1, [2, 3], False)

                # oG ci 4,5 from the head-2 exchange
                for r2 in range(2):
                    nc.sync.dma_start(
                        oG[:, 4:6, :].rearrange("(r2 s) o w -> r2 s o w", s=DH)[r2],
                        a2a2_out[0:NJ, :, :].rearrange(
                            "(o r2) s w -> r2 s o w", r2=2
                        )[r2],
                    )
                oGt2 = rows.tile([P, 2, QS], bf16, tag="ogt2", bufs=1, name="oGt2")
                for r2 in range(2):
                    nc.sync.dma_start(
                        oGt2[:].rearrange("(r2 s) o w -> r2 s o w", s=DH)[r2],
                        a2a2_out[NJ : 2 * NJ, :, :].rearrange(
                            "(o r2) s w -> r2 s o w", r2=2
                        )[r2],
                    )
                nc.vector.tensor_scalar(
                    oG[:, 4:6, :], oG[:, 4:6, :], s0b[:], None, Alu.mult
                )
                nc.vector.tensor_scalar(oGt2[:], oGt2[:], s1b[:], None, Alu.mult)
                nc.vector.tensor_tensor(oG[:, 4:6, :], oG[:, 4:6, :], oGt2[:], Alu.add)

                # ---- tail: finish out-proj, residual + LayerNorm, store ----
                for qt in range(QT):
                    if qt < 2:
                        d_partial(qt, [4, 5], False)
                    else:
                        d_partial(qt, list(range(CI)), True)
                    x1 = ppool.tile([P, C], f32, tag="x1", bufs=2, name="x1")
                    nc.vector.tensor_tensor(x1[:], px[qt][:], qres_sb[:, qt], Alu.add)
                    del px[qt]
                    mu = small.tile([P, 1], f32, tag="st", name="mu")
                    sq_t = ppool.tile([P, C], f32, tag="sq", bufs=2, name="sq")
                    sqs = small.tile([P, 1], f32, tag="st", name="sqs")
                    var = small.tile([P, 1], f32, tag="st", name="var")
                    rinv = small.tile([P, 1], f32, tag="st", name="rinv")
                    rstd = small.tile([P, 1], f32, tag="st", name="rstd")
                    nb = small.tile([P, 1], f32, tag="st", name="nb")
                    nc.vector.reduce_sum(mu[:], x1[:], axis=mybir.AxisListType.X)
                    nc.vector.tensor_scalar_mul(mu[:], mu[:], 1.0 / C)
                    nc.scalar.activation(sq_t[:], x1[:], Act.Square, accum_out=sqs[:])
                    nc.vector.tensor_scalar_mul(sqs[:], sqs[:], 1.0 / C)
                    nc.vector.tensor_tensor(var[:], mu[:], mu[:], Alu.mult)
                    nc.vector.tensor_tensor(var[:], sqs[:], var[:], Alu.subtract)
                    nc.vector.tensor_scalar_add(var[:], var[:], EPS)
                    nc.vector.reciprocal(rinv[:], var[:])
                    nc.scalar.activation(rstd[:], rinv[:], Act.Sqrt)
                    nc.vector.tensor_tensor(nb[:], mu[:], rstd[:], Alu.mult)
                    nc.vector.tensor_scalar_mul(nb[:], nb[:], -1.0)
                    nc.vector.tensor_scalar(
                        x1[:], x1[:], rstd[:], nb[:], Alu.mult, Alu.add
                    )
                    nc.vector.tensor_tensor(x1[:], x1[:], gamma_b[:], Alu.mult)
                    nc.vector.tensor_tensor(x1[:], x1[:], beta_b[:], Alu.add)
                    nc.sync.dma_start(
                        y.rearrange("(t p) c -> p t c", p=P)[:, qt], x1[:]
                    )

        for pool in (dram, small, ppool, rows, persist, const):
            pool.release()

    nc.compile()
    return nc


def get_nc():
    if "nc" not in _NC_CACHE:
        _NC_CACHE["nc"] = _build_nc()
    return _NC_CACHE["nc"]


def make_in_maps(inputs):
    import ml_dtypes

    b16 = ml_dtypes.bfloat16
    q = np.asarray(inputs["query"], np.float32)
    k = np.asarray(inputs["key_in"], np.float32)
    v = np.asarray(inputs["value"], np.float32)
    Wq = np.asarray(inputs["Wq"], np.float32)
    Wk = np.asarray(inputs["Wk"], np.float32)
    Wv = np.asarray(inputs["Wv"], np.float32)
    Wo = np.asarray(inputs["Wo"], np.float32)
    bq = np.asarray(inputs["bq"], np.float32)
    bk = np.asarray(inputs["bk"], np.float32)
    bv = np.asarray(inputs["bv"], np.float32)
    bo = np.asarray(inputs["bo"], np.float32)
    gamma = np.asarray(inputs["gamma"], np.float32)
    beta = np.asarray(inputs["beta"], np.float32)

    in_maps = []
    for c in range(NCORES):
        b, g = c // 4, c % 4
        cs = slice(CS * g, CS * (g + 1))
        in_maps.append(
            {
                "qT": np.ascontiguousarray(q[b].T).astype(b16),
                "kT": np.ascontiguousarray(k[b].T).astype(b16),
                "vT": np.ascontiguousarray(v[b].T).astype(b16),
                "wq": np.ascontiguousarray(Wq[:, cs]).astype(b16),
                "wk": np.ascontiguousarray(Wk[:, cs]).astype(b16),
                "wv": np.ascontiguousarray(Wv[:, cs]).astype(b16),
                "wo": Wo[WO_PERM, :].astype(b16),
                "bq": np.ascontiguousarray(bq[cs]),
                "bk": np.ascontiguousarray(bk[cs]),
                "bv": np.ascontiguousarray(bv[cs]),
                "bo": bo.copy(),
                "gamma": gamma.copy(),
                "beta": beta.copy(),
                "qres": np.ascontiguousarray(q[b, QS * g : QS * (g + 1)]),
                "gsel": np.array([1.0 - b, float(b)], np.float32),
            }
        )
    return in_maps


def _install_ntff_shim():
    """Provide antenv.axon_hooks if the image lacks it (needed for trace=True)."""
    try:
        import antenv.axon_hooks  # noqa: F401

        return
    except ImportError:
        pass
    import contextlib
    import ctypes
    import types

    so_path = "/opt/axon/libaxon_pjrt.so"
    state = {"hook": None}

    def set_axon_ntff_profile_hook(h):
        state["hook"] = h

    def get_axon_ntff_profile_hook():
        if state["hook"] is None:
            try:
                lib = ctypes.CDLL(so_path)
            except OSError:
                return None
            if not hasattr(lib, "axon_start_nrt_profile"):
                return None
            lib.axon_start_nrt_profile.argtypes = [
                ctypes.POINTER(ctypes.c_int64),
                ctypes.c_size_t,
            ]
            lib.axon_start_nrt_profile.restype = ctypes.c_int64
            lib.axon_stop_nrt_profile.argtypes = [ctypes.c_char_p]
            lib.axon_stop_nrt_profile.restype = ctypes.c_int64

            @contextlib.contextmanager
            def _hook(output_dir, device_ids):
                import jax

                jax.devices()
                if device_ids:
                    ids = (ctypes.c_int64 * len(device_ids))(*device_ids)
                    rc = lib.axon_start_nrt_profile(ids, len(device_ids))
                else:
                    rc = lib.axon_start_nrt_profile(None, 0)
                if rc != 0:
                    raise RuntimeError(f"axon_start_nrt_profile rc={rc}")
                try:
                    yield
                finally:
                    n = lib.axon_stop_nrt_profile(str(output_dir).encode())
                    print(f"profile: {n} file(s) written to {output_dir}")

            state["hook"] = _hook
        return state["hook"]

    mod = types.ModuleType("antenv.axon_hooks")
    mod.set_axon_ntff_profile_hook = set_axon_ntff_profile_hook
    mod.get_axon_ntff_profile_hook = get_axon_ntff_profile_hook
    import antenv

    antenv.axon_hooks = mod
    sys.modules["antenv.axon_hooks"] = mod


def run(inputs, trace=False, trace_cores=None):
    if trace:
        _install_ntff_shim()
    from concourse.bass_utils import run_bass_kernel_spmd

    nc = get_nc()
    in_maps = make_in_maps(inputs)
    res = run_bass_kernel_spmd(
        nc,
        in_maps,
        list(range(NCORES)),
        trace=trace,
        **({"trace_cores": trace_cores} if trace_cores is not None else {}),
    )
    out = np.empty((B, N, C), np.float32)
    for c in range(NCORES):
        b, g = c // 4, c % 4
        out[b, QS * g : QS * (g + 1)] = res.results[c]["y"]
    return out, res


def kernel(**inputs):
    out, _ = run(inputs, trace=False)
    return out


# revision 9
# speedup vs baseline: 1.1861x; 1.1400x over previous
"""Trainium2 Bass kernel for CrossAttention (B=2, N=2048, C=768, H=12).

Sharding: core c -> batch b=c//4, head-group g=c%4 (3 heads each).
Each core computes Q/K/V projections for its heads over the full sequence and
attention; an AllToAll exchanges per-head outputs so each core then computes
the full output projection, residual and LayerNorm for its own 512-row
q-shard.

v2 schedule: single fused region.  K-proj warms the PE, Q-proj is emitted
just-in-time per 512-column q-chunk and V-proj just-in-time per kv-block so
projection matmuls fill the PE bubbles of the scalar(exp)-bound attention
loop (keeps the HAM clock-gate at 8/8).  The AllToAll is split in two:
heads {0,1} fire after their attention finishes and transfer under head-2's
attention; only the small head-2 AllToAll plus the tail of the output
projection is exposed.

kernel(**inputs) takes the FULL inputs (setup_inputs() keys) and returns the
full [2, 2048, 768] output.
"""

import sys

for _p in ("/opt/trn_rl_repo",):
    if _p not in sys.path:
        sys.path.insert(0, _p)

import numpy as np

B, N, C = 2, 2048, 768
H = 12
DH = 64
EPS = 1e-5
SCALE = DH ** (-0.5)  # 0.125

NCORES = 8
HPC = 3          # heads per core
CS = HPC * DH    # 192 output-feature slice per core
QS = N // 4      # 512 q rows per core
P = 128

_NC_CACHE = {}

# Wo row permutation: gathered order is [per-group heads (3g, 3g+1)] then
# [per-group head 3g+2]; Wo rows must match.
import numpy as _np
WO_PERM = _np.concatenate(
    [_np.arange(192 * g, 192 * g + 128) for g in range(4)]
    + [_np.arange(192 * g + 128, 192 * (g + 1)) for g in range(4)]
)


def _build_nc():
    import concourse.bass as bass
    import concourse.mybir as mybir
    import concourse.tile as tile
    from concourse import bacc

    f32 = mybir.dt.float32
    bf16 = mybir.dt.bfloat16
    Alu = mybir.AluOpType
    Act = mybir.ActivationFunctionType

    nc = bacc.Bacc(
        "TRN2",
        target_bir_lowering=False,
        debug=False,
        enable_asserts=True,
        num_devices=NCORES,
    )

    # ---- kernel I/O (per-core shapes; host shards the full problem) ----
    qT = nc.dram_tensor("qT", [C, N], bf16, kind="ExternalInput").ap()
    kT = nc.dram_tensor("kT", [C, N], bf16, kind="ExternalInput").ap()
    vT = nc.dram_tensor("vT", [C, N], bf16, kind="ExternalInput").ap()
    wq = nc.dram_tensor("wq", [C, CS], bf16, kind="ExternalInput").ap()
    wk = nc.dram_tensor("wk", [C, CS], bf16, kind="ExternalInput").ap()
    wv = nc.dram_tensor("wv", [C, CS], bf16, kind="ExternalInput").ap()
    wo = nc.dram_tensor("wo", [C, C], bf16, kind="ExternalInput").ap()
    bq = nc.dram_tensor("bq", [CS], f32, kind="ExternalInput").ap()
    bk = nc.dram_tensor("bk", [CS], f32, kind="ExternalInput").ap()
    bv = nc.dram_tensor("bv", [CS], f32, kind="ExternalInput").ap()
    gamma = nc.dram_tensor("gamma", [C], f32, kind="ExternalInput").ap()
    beta = nc.dram_tensor("beta", [C], f32, kind="ExternalInput").ap()
    qres = nc.dram_tensor("qres", [QS, C], f32, kind="ExternalInput").ap()
    gsel = nc.dram_tensor("gsel", [2], f32, kind="ExternalInput").ap()
    y = nc.dram_tensor("y", [QS, C], f32, kind="ExternalOutput").ap()

    CI = C // P          # 6 contraction chunks
    NJ = N // 512        # 4 q-chunks of 512
    NM = N // P          # 16 kv-chunks of 128
    VS = DH + 1          # 65: v columns + ones column (denominator row)
    QT = QS // P         # 4 output row-blocks of 128

    with tile.TileContext(nc) as tc:
        const = tc.alloc_tile_pool(name="const", bufs=1)
        persist = tc.alloc_tile_pool(name="persist", bufs=1)
        rows = tc.alloc_tile_pool(name="rows", bufs=2)
        ppool = tc.alloc_tile_pool(name="ppool", bufs=3)
        small = tc.alloc_tile_pool(name="small", bufs=4)
        dram = tc.alloc_tile_pool(name="dram", bufs=1, space="DRAM")

        # ---------- weights + j-blocked input DMAs (priority order) -------
        wk_sb = const.tile([P, CI, CS], bf16, name="wk_sb")
        nc.sync.dma_start(wk_sb[:], wk.rearrange("(o p) m -> p o m", p=P))
        bkA = const.tile([P, 1], f32, name="bkA")
        bkB = const.tile([DH, 1], f32, name="bkB")
        nc.sync.dma_start(bkA[:], bk[0:P][:, None])
        nc.sync.dma_start(bkB[:], bk[P:CS][:, None])
        wq_sb = const.tile([P, CI, CS], bf16, name="wq_sb")
        nc.sync.dma_start(wq_sb[:], wq.rearrange("(o p) m -> p o m", p=P))
        bqA = const.tile([P, 1], f32, name="bqA")
        bqB = const.tile([DH, 1], f32, name="bqB")
        nc.sync.dma_start(bqA[:], bq[0:P][:, None])
        nc.sync.dma_start(bqB[:], bq[P:CS][:, None])
        wv_sb = const.tile([P, CI, CS], bf16, name="wv_sb")
        nc.sync.dma_start(wv_sb[:], wv.rearrange("(o p) m -> p o m", p=P))
        bv_b = const.tile([P, CS], f32, name="bv_b")
        nc.sync.dma_start(bv_b[0:1, :], bv[None, :])
        nc.gpsimd.partition_broadcast(bv_b[:], bv_b[0:1, :])

        k_rows = [
            rows.tile([P, N], bf16, tag="krow", bufs=6, name=f"k_row{i}")
            for i in range(CI)
        ]
        q_rows = [persist.tile([P, N], bf16, name=f"q_row{i}") for i in range(CI)]
        v_rows = [
            rows.tile([P, N], bf16, tag="vrow", bufs=6, name=f"v_row{i}")
            for i in range(CI)
        ]
        # j-block 0 of kT, then q columns 0:512, then vT block 0, then the
        # rest round-robin so compute can chase the DMA stream
        for i in range(CI):
            nc.sync.dma_start(k_rows[i][:, 0:512], kT[P * i : P * (i + 1), 0:512])
        for i in range(CI):
            nc.sync.dma_start(q_rows[i][:, 0:512], qT[P * i : P * (i + 1), 0:512])
        for i in range(CI):
            nc.sync.dma_start(v_rows[i][:, 0:512], vT[P * i : P * (i + 1), 0:512])
        for j in range(1, NJ):
            s5 = slice(512 * j, 512 * (j + 1))
            for i in range(CI):
                nc.sync.dma_start(k_rows[i][:, s5], kT[P * i : P * (i + 1), s5])
            for i in range(CI):
                nc.sync.dma_start(v_rows[i][:, s5], vT[P * i : P * (i + 1), s5])
        for i in range(CI):
            nc.sync.dma_start(q_rows[i][:, 512:N], qT[P * i : P * (i + 1), 512:N])

        # ---------- persistent activations ----------
        qTa = persist.tile([P, N], bf16, name="qTa")    # heads 0,1
        qTb = persist.tile([DH, N], bf16, name="qTb")   # head 2
        kTa = persist.tile([P, N], bf16, name="kTa")
        kTb = persist.tile([DH, N], bf16, name="kTb")
        vaug = persist.tile([P, NM, HPC * VS], bf16, name="vaug")
        nc.vector.memset(
            vaug.rearrange("p m (h d) -> p m h d", d=VS)[:, :, :, DH : DH + 1], 1.0
        )
        o_hb = [persist.tile([DH, N], bf16, name=f"ob{h}") for h in range(HPC)]
        oG = persist.tile([P, CI, QS], bf16, name="oG")

        wo_sb = const.tile([P, CI, C], bf16, name="wo_sb")
        gs = const.tile([1, 2], f32, name="gs")
        s0b = const.tile([P, 1], f32, name="s0b")
        s1b = const.tile([P, 1], f32, name="s1b")
        gamma_b = const.tile([P, C], f32, name="gamma_b")
        beta_b = const.tile([P, C], f32, name="beta_b")
        qres_sb = const.tile([P, QT, C], f32, name="qres_sb")

        a2a1_in = dram.tile([2 * NJ, P, QS], bf16, name="a2a1_in")
        a2a1_out = dram.tile([2 * NJ, P, QS], bf16, name="a2a1_out")
        a2a2_in = dram.tile([2 * NJ, DH, QS], bf16, name="a2a2_in")
        a2a2_out = dram.tile([2 * NJ, DH, QS], bf16, name="a2a2_out")

        # =========== fused projections + attention (heads 0,1) ===========
        with (
            tc.tile_pool(name="ppO", bufs=1, space="PSUM") as ppO,
            tc.tile_pool(name="ppS", bufs=2, space="PSUM") as ppS,
            tc.tile_pool(name="ppF", bufs=2, space="PSUM") as ppF,
        ):
            po_h = {
                0: ppO.tile([P, 512], f32, tag="po0", name="po0"),
                1: ppO.tile([P, 512], f32, tag="po1", name="po1"),
            }

            def k_proj(j):
                s5 = slice(512 * j, 512 * (j + 1))
                pk_a = ppF.tile([P, 512], f32, tag="fill", name=f"pka{j}")
                for i in range(CI):
                    nc.tensor.matmul(
                        pk_a[:], wk_sb[:, i, 0:P], k_rows[i][:, s5],
                        start=(i == 0), stop=(i == CI - 1),
                    )
                nc.vector.tensor_tensor(
                    kTa[:, s5], pk_a[:], bkA.to_broadcast((P, 512)), Alu.add
                )
                pk_b = ppF.tile([P, 512], f32, tag="fill", name=f"pkb{j}")
                for i in range(CI):
                    nc.tensor.matmul(
                        pk_b[0:DH], wk_sb[:, i, P:CS], k_rows[i][:, s5],
                        start=(i == 0), stop=(i == CI - 1),
                    )
                nc.vector.tensor_tensor(
                    kTb[:, s5], pk_b[0:DH], bkB.to_broadcast((DH, 512)), Alu.add
                )

            def q_proj(r):
                s5 = slice(512 * r, 512 * (r + 1))
                pq_a = ppF.tile([P, 512], f32, tag="fill", name=f"pqa{r}")
                for i in range(CI):
                    nc.tensor.matmul(
                        pq_a[:], wq_sb[:, i, 0:P], q_rows[i][:, s5],
                        start=(i == 0), stop=(i == CI - 1),
                    )
                nc.vector.tensor_tensor(
                    qTa[:, s5], pq_a[:], bqA.to_broadcast((P, 512)), Alu.add
                )
                pq_b = ppF.tile([P, 512], f32, tag="fill", name=f"pqb{r}")
                for i in range(CI):
                    nc.tensor.matmul(
                        pq_b[0:DH], wq_sb[:, i, P:CS], q_rows[i][:, s5],
                        start=(i == 0), stop=(i == CI - 1),
                    )
                nc.vector.tensor_tensor(
                    qTb[:, s5], pq_b[0:DH], bqB.to_broadcast((DH, 512)), Alu.add
                )

            def v_proj(m):
                pv = ppF.tile([P, 512], f32, tag="fill", name=f"pv{m}")
                for i in range(CI):
                    nc.tensor.matmul(
                        pv[:, 0:CS], v_rows[i][:, P * m : P * (m + 1)], wv_sb[:, i, :],
                        start=(i == 0), stop=(i == CI - 1),
                    )
                dst = vaug.rearrange("p m (h d) -> p m h d", d=VS)[:, m, :, 0:DH]
                nc.vector.tensor_tensor(
                    dst,
                    pv[:, 0:CS].rearrange("p (h d) -> p h d", d=DH),
                    bv_b.rearrange("p (h d) -> p h d", d=DH),
                    Alu.add,
                )

            def evict_head(h, r, po):
                s5 = slice(512 * r, 512 * (r + 1))
                l_t = small.tile([1, 512], f32, tag="lt", name=f"l{h}{r}")
                nc.vector.tensor_copy(l_t[:], po[DH : DH + 1, :])
                r_t = small.tile([1, 512], f32, tag="lt", name=f"rr{h}{r}")
                nc.vector.reciprocal_approx_fast(out=r_t[:], in_=l_t[:])
                rb = ppool.tile([DH, 512], f32, tag="rb", bufs=2, name=f"rb{h}{r}")
                nc.gpsimd.partition_broadcast(rb[:], r_t[:])
                nc.vector.tensor_tensor(o_hb[h][:, s5], po[0:DH, :], rb[:], Alu.mult)

            # projection lead-in: K j=0, Q r=0 chase the first DMA blocks
            k_proj(0)
            q_proj(0)
            v_proj(0)
            v_proj(1)

            # flat software-pipelined loop over (r, m); av trails by 1 step
            steps = [(r, m) for r in range(NJ) for m in range(NM)]
            pts = {}
            for si, (r, m) in enumerate(steps):
                sq = slice(512 * r, 512 * (r + 1))
                sm = slice(P * m, P * (m + 1))
                s_t = ppS.tile([P, 1024], f32, tag="s", name=f"s{r}_{m}")
                nc.tensor.matmul(
                    s_t[:, 0:512], kTa[0:DH, sm], qTa[0:DH, sq],
                    start=True, stop=True,
                )
                nc.tensor.matmul(
                    s_t[:, 512:1024], kTa[DH:P, sm], qTa[DH:P, sq],
                    start=True, stop=True,
                )
                pt = ppool.tile([P, 1024], bf16, tag="p", bufs=4, name="pt")
                nc.scalar.activation(pt[:], s_t[:], Act.Exp, scale=SCALE)
                pts[si] = pt
                # PE fillers while exp runs: rest of K-proj, V-proj, Q-proj
                if r == 0:
                    if m in (1, 5, 9):
                        k_proj(m // 4 + 1)
                    if m + 2 < NM:
                        v_proj(m + 2)
                if r < NJ - 1 and m == 8:
                    q_proj(r + 1)
                if si >= 1:
                    pr, pm = steps[si - 1]
                    for h in (0, 1):
                        nc.tensor.matmul(
                            po_h[h][0:VS],
                            vaug[:, pm, VS * h : VS * (h + 1)],
                            pts[si - 1][:, 512 * h : 512 * (h + 1)],
                            start=(pm == 0), stop=(pm == NM - 1),
                        )
                    del pts[si - 1]
                    if pm == NM - 1:
                        for h in (0, 1):
                            evict_head(h, pr, po_h[h])
                        for h in (0, 1):
                            for g2 in range(2):
                                nc.sync.dma_start(
                                    a2a1_in[NJ * g2 + pr, DH * h : DH * (h + 1), :],
                                    o_hb[h][:, 512 * pr : 512 * (pr + 1)],
                                )
            (r, m) = steps[-1]
            for h in (0, 1):
                nc.tensor.matmul(
                    po_h[h][0:VS],
                    vaug[:, m, VS * h : VS * (h + 1)],
                    pts[len(steps) - 1][:, 512 * h : 512 * (h + 1)],
                    start=False, stop=True,
                )
            for h in (0, 1):
                evict_head(h, r, po_h[h])
            for h in (0, 1):
                for g2 in range(2):
                    nc.sync.dma_start(
                        a2a1_in[NJ * g2 + r, DH * h : DH * (h + 1), :],
                        o_hb[h][:, 512 * r : 512 * (r + 1)],
                    )

            nc.gpsimd.collective_compute(
                "AllToAll",
                Alu.bypass,
                replica_groups=[list(range(NCORES))],
                ins=[a2a1_in.opt()],
                outs=[a2a1_out.opt()],
            )

            # tail-only constants (kept off the startup DMA queues)
            nc.sync.dma_start(wo_sb[:], wo.rearrange("(o p) m -> p o m", p=P))
            nc.sync.dma_start(gs[:], gsel[None, :])
            nc.gpsimd.partition_broadcast(s0b[:], gs[0:1, 0:1])
            nc.gpsimd.partition_broadcast(s1b[:], gs[0:1, 1:2])
            nc.sync.dma_start(gamma_b[0:1, :], gamma[None, :])
            nc.sync.dma_start(beta_b[0:1, :], beta[None, :])
            nc.gpsimd.partition_broadcast(gamma_b[:], gamma_b[0:1, :])
            nc.gpsimd.partition_broadcast(beta_b[:], beta_b[0:1, :])
            nc.sync.dma_start(qres_sb[:], qres.rearrange("(t p) c -> p t c", p=P))

            # ---------------- head 2 ----------------
            h2 = 2
            po2 = ppO.tile([P, 512], f32, tag="po0", name="po2")
            h2_steps = [(r, mp) for r in range(NJ) for mp in range(NM // 2)]
            pt2s = {}
            for si, (r, mp) in enumerate(h2_steps):
                sq = slice(512 * r, 512 * (r + 1))
                s_t = ppS.tile([P, 1024], f32, tag="s", name=f"t{r}_{mp}")
                for q2 in range(2):
                    m = 2 * mp + q2
                    nc.tensor.matmul(
                        s_t[:, 512 * q2 : 512 * (q2 + 1)],
                        kTb[0:DH, P * m : P * (m + 1)],
                        qTb[0:DH, sq],
                        start=True, stop=True,
                    )
                pt2 = ppool.tile([P, 1024], bf16, tag="p", bufs=4, name="pt2")
                nc.scalar.activation(pt2[:], s_t[:], Act.Exp, scale=SCALE)
                pt2s[si] = pt2
                if si >= 1:
                    pr, pmp = h2_steps[si - 1]
                    for q2 in range(2):
                        m = 2 * pmp + q2
                        nc.tensor.matmul(
                            po2[0:VS],
                            vaug[:, m, VS * h2 : VS * (h2 + 1)],
                            pt2s[si - 1][:, 512 * q2 : 512 * (q2 + 1)],
                            start=(m == 0), stop=(m == NM - 1),
                        )
                    del pt2s[si - 1]
                    if pmp == NM // 2 - 1:
                        evict_head(2, pr, po2)
                        for g2 in range(2):
                            nc.sync.dma_start(
                                a2a2_in[NJ * g2 + pr, :, :],
                                o_hb[2][:, 512 * pr : 512 * (pr + 1)],
                            )
            (r, mp) = h2_steps[-1]
            for q2 in range(2):
                m = 2 * mp + q2
                nc.tensor.matmul(
                    po2[0:VS],
                    vaug[:, m, VS * h2 : VS * (h2 + 1)],
                    pt2s[len(h2_steps) - 1][:, 512 * q2 : 512 * (q2 + 1)],
                    start=False, stop=(m == NM - 1),
                )
            evict_head(2, r, po2)
            for g2 in range(2):
                nc.sync.dma_start(
                    a2a2_in[NJ * g2 + r, :, :], o_hb[2][:, 512 * r : 512 * (r + 1)]
                )

        nc.gpsimd.collective_compute(
            "AllToAll",
            Alu.bypass,
            replica_groups=[list(range(NCORES))],
            ins=[a2a2_in.opt()],
            outs=[a2a2_out.opt()],
        )

        # ========== output projection + residual + LayerNorm ==========
        with tc.tile_pool(name="ppD", bufs=4, space="PSUM") as ppD:
            # assemble oG ci 0..3 from the heads-0,1 exchange (done long ago)
            oGt1 = rows.tile([P, NJ, QS], bf16, tag="krow", bufs=6, name="oGt1")
            nc.sync.dma_start(
                oG[:, 0:NJ, :], a2a1_out[0:NJ, :, :].rearrange("r s w -> s r w")
            )
            nc.sync.dma_start(
                oGt1[:], a2a1_out[NJ : 2 * NJ, :, :].rearrange("r s w -> s r w")
            )
            nc.vector.tensor_scalar(
                oG[:, 0:NJ, :], oG[:, 0:NJ, :], s0b[:], None, Alu.mult
            )
            nc.vector.tensor_scalar(oGt1[:], oGt1[:], s1b[:], None, Alu.mult)
            nc.vector.tensor_tensor(oG[:, 0:NJ, :], oG[:, 0:NJ, :], oGt1[:], Alu.add)

            px = {}
            def d_partial(qt, ci_list, start_first):
                if qt not in px:
                    px[qt] = ppD.tile([P, C], f32, tag="px", name=f"px{qt}")
                for idx, ci in enumerate(ci_list):
                    st = dict(start=(start_first and idx == 0), stop=(ci == CI - 1))
                    nc.tensor.matmul(
                        px[qt][:, 0:512],
                        oG[:, ci, P * qt : P * (qt + 1)],
                        wo_sb[:, ci, 0:512],
                        **st,
                    )
                    nc.tensor.matmul(
                        px[qt][:, 512:C],
                        oG[:, ci, P * qt : P * (qt + 1)],
                        wo_sb[:, ci, 512:C],
                        **st,
                    )

            # out-proj over the already-received 2/3 of the contraction
            # (overlaps the head-2 AllToAll)
            for qt in range(QT):
                d_partial(qt, [0, 1, 2, 3], True)

            # oG ci 4,5 from the head-2 exchange
            for r2 in range(2):
                nc.sync.dma_start(
                    oG[:, 4:6, :].rearrange("(r2 s) o w -> r2 s o w", s=DH)[r2],
                    a2a2_out[0:NJ, :, :].rearrange("(o r2) s w -> r2 s o w", r2=2)[r2],
                )
            oGt2 = rows.tile([P, 2, QS], bf16, tag="ogt2", bufs=1, name="oGt2")
            for r2 in range(2):
                nc.sync.dma_start(
                    oGt2[:].rearrange("(r2 s) o w -> r2 s o w", s=DH)[r2],
                    a2a2_out[NJ : 2 * NJ, :, :].rearrange(
                        "(o r2) s w -> r2 s o w", r2=2
                    )[r2],
                )
            nc.vector.tensor_scalar(
                oG[:, 4:6, :], oG[:, 4:6, :], s0b[:], None, Alu.mult
            )
            nc.vector.tensor_scalar(oGt2[:], oGt2[:], s1b[:], None, Alu.mult)
            nc.vector.tensor_tensor(oG[:, 4:6, :], oG[:, 4:6, :], oGt2[:], Alu.add)

            for qt in range(QT):
                d_partial(qt, [4, 5], False)
                # x1 = out-proj + residual (qres has bo folded in host-side)
                x1 = ppool.tile([P, C], f32, tag="x1", bufs=2, name="x1")
                nc.vector.tensor_tensor(x1[:], px[qt][:], qres_sb[:, qt], Alu.add)
                musum = small.tile([P, 1], f32, tag="st", name="musum")
                mu = small.tile([P, 1], f32, tag="st", name="mu")
                sq_t = ppool.tile([P, C], f32, tag="sq", bufs=2, name="sq")
                sqs = small.tile([P, 1], f32, tag="st", name="sqs")
                var = small.tile([P, 1], f32, tag="st", name="var")
                rinv = small.tile([P, 1], f32, tag="st", name="rinv")
                rstd = small.tile([P, 1], f32, tag="st", name="rstd")
                nb = small.tile([P, 1], f32, tag="st", name="nb")
                nc.vector.reduce_sum(musum[:], x1[:], axis=mybir.AxisListType.X)
                nc.vector.tensor_scalar_mul(mu[:], musum[:], 1.0 / C)
                nc.scalar.activation(sq_t[:], x1[:], Act.Square, accum_out=sqs[:])
                nc.vector.tensor_tensor(var[:], mu[:], mu[:], Alu.mult)
                nc.vector.scalar_tensor_tensor(
                    var[:], sqs[:], 1.0 / C, var[:], Alu.mult, Alu.subtract
                )
                nc.vector.tensor_scalar_add(var[:], var[:], EPS)
                nc.vector.reciprocal(rinv[:], var[:])
                nc.scalar.activation(rstd[:], rinv[:], Act.Sqrt)
                nc.vector.scalar_tensor_tensor(
                    nb[:], mu[:], -1.0, rstd[:], Alu.mult, Alu.mult
                )
                nc.vector.tensor_scalar(
                    x1[:], x1[:], rstd[:], nb[:], Alu.mult, Alu.add
                )
                nc.gpsimd.tensor_tensor(x1[:], x1[:], gamma_b[:], Alu.mult)
                nc.vector.tensor_tensor(x1[:], x1[:], beta_b[:], Alu.add)
                nc.sync.dma_start(
                    y.rearrange("(t p) c -> p t c", p=P)[:, qt], x1[:]
                )
                del px[qt]

        for pool in (dram, small, ppool, rows, persist, const):
            pool.release()

    nc.compile()
    return nc


def get_nc():
    if "nc" not in _NC_CACHE:
        _NC_CACHE["nc"] = _build_nc()
    return _NC_CACHE["nc"]


def make_in_maps(inputs):
    import ml_dtypes

    b16 = ml_dtypes.bfloat16
    q = np.asarray(inputs["query"], np.float32)
    k = np.asarray(inputs["key_in"], np.float32)
    v = np.asarray(inputs["value"], np.float32)
    Wq = np.asarray(inputs["Wq"], np.float32)
    Wk = np.asarray(inputs["Wk"], np.float32)
    Wv = np.asarray(inputs["Wv"], np.float32)
    Wo = np.asarray(inputs["Wo"], np.float32)
    bq = np.asarray(inputs["bq"], np.float32)
    bk = np.asarray(inputs["bk"], np.float32)
    bv = np.asarray(inputs["bv"], np.float32)
    bo = np.asarray(inputs["bo"], np.float32)
    gamma = np.asarray(inputs["gamma"], np.float32)
    beta = np.asarray(inputs["beta"], np.float32)

    in_maps = []
    for c in range(NCORES):
        b, g = c // 4, c % 4
        cs = slice(CS * g, CS * (g + 1))
        in_maps.append(
            {
                "qT": np.ascontiguousarray(q[b].T).astype(b16),
                "kT": np.ascontiguousarray(k[b].T).astype(b16),
                "vT": np.ascontiguousarray(v[b].T).astype(b16),
                "wq": np.ascontiguousarray(Wq[:, cs]).astype(b16),
                "wk": np.ascontiguousarray(Wk[:, cs]).astype(b16),
                "wv": np.ascontiguousarray(Wv[:, cs]).astype(b16),
                "wo": Wo[WO_PERM, :].astype(b16),
                "bq": np.ascontiguousarray(bq[cs]),
                "bk": np.ascontiguousarray(bk[cs]),
                "bv": np.ascontiguousarray(bv[cs]),
                "gamma": gamma.copy(),
                "beta": beta.copy(),
                "qres": np.ascontiguousarray(q[b, QS * g : QS * (g + 1)] + bo[None, :]),
                "gsel": np.array([1.0 - b, float(b)], np.float32),
            }
        )
    return in_maps


def _install_ntff_shim():
    """Provide antenv.axon_hooks if the image lacks it (needed for trace=True)."""
    try:
        import antenv.axon_hooks  # noqa: F401

        return
    except ImportError:
        pass
    import contextlib
    import ctypes
    import types

    so_path = "/opt/axon/libaxon_pjrt.so"
    state = {"hook": None}

    def set_axon_ntff_profile_hook(h):
        state["hook"] = h

    def get_axon_ntff_profile_hook():
        if state["hook"] is None:
            try:
                lib = ctypes.CDLL(so_path)
            except OSError:
                return None
            if not hasattr(lib, "axon_start_nrt_profile"):
                return None
            lib.axon_start_nrt_profile.argtypes = [
                ctypes.POINTER(ctypes.c_int64),
                ctypes.c_size_t,
            ]
            lib.axon_start_nrt_profile.restype = ctypes.c_int64
            lib.axon_stop_nrt_profile.argtypes = [ctypes.c_char_p]
            lib.axon_stop_nrt_profile.restype = ctypes.c_int64

            @contextlib.contextmanager
            def _hook(output_dir, device_ids):
                import jax

                jax.devices()
                if device_ids:
                    ids = (ctypes.c_int64 * len(device_ids))(*device_ids)
                    rc = lib.axon_start_nrt_profile(ids, len(device_ids))
                else:
                    rc = lib.axon_start_nrt_profile(None, 0)
                if rc != 0:
                    raise RuntimeError(f"axon_start_nrt_profile rc={rc}")
                try:
                    yield
                finally:
                    n = lib.axon_stop_nrt_profile(str(output_dir).encode())
                    print(f"profile: {n} file(s) written to {output_dir}")

            state["hook"] = _hook
        return state["hook"]

    mod = types.ModuleType("antenv.axon_hooks")
    mod.set_axon_ntff_profile_hook = set_axon_ntff_profile_hook
    mod.get_axon_ntff_profile_hook = get_axon_ntff_profile_hook
    import antenv

    antenv.axon_hooks = mod
    sys.modules["antenv.axon_hooks"] = mod


def run(inputs, trace=False, trace_cores=None):
    if trace:
        _install_ntff_shim()
    from concourse.bass_utils import run_bass_kernel_spmd

    nc = get_nc()
    in_maps = make_in_maps(inputs)
    res = run_bass_kernel_spmd(
        nc,
        in_maps,
        list(range(NCORES)),
        trace=trace,
        **({"trace_cores": trace_cores} if trace_cores is not None else {}),
    )
    out = np.empty((B, N, C), np.float32)
    for c in range(NCORES):
        b, g = c // 4, c % 4
        out[b, QS * g : QS * (g + 1)] = res.results[c]["y"]
    return out, res


def kernel(**inputs):
    out, _ = run(inputs, trace=False)
    return out
